# revision 17
# baseline (speedup 1.0000x reference)
"""Trainium2 Bass kernel: single-step attention decoder RNN (AttnDecoderRNN).

Contract: kernel(**inputs) takes the FULL unsharded inputs (same keys as the
reference setup_inputs) and returns the FULL output pytree:
    (logprobs[1,V], (hidden[2,1,H], cell[2,1,H]), attn_weights[1,L])

Sharding (8 NeuronCores, SPMD):
  - attention + W_comb projection: replicated compute on every core (bf16
    weights), so no collective is needed before the LSTM
  - LSTM: f32; each core owns a 128-wide slice of each gate (i,f,g,o); full h
    is re-assembled with an AllGather between layers
  - W_out / b_out: bf16 / f32, sharded along vocab (6283 rows per core,
    padded to 6656); log-softmax denominator combined with an AllGather of
    per-core partial sums of exp(logits)
All weight matrices are transposed host-side so the contraction dim lands on
SBUF partitions (PE matmul contracts along partitions). Activation vectors
live as [128,1] partition columns where they feed contractions and as [1,N]
rows where pointwise math happens; row->column flips use PE transposes.
A dummy AllGather at kernel start absorbs the ncfw cold-start latency.
"""

import numpy as np

H = 1024
V = 50257
L = 256
NCORES = 8
HS = H // NCORES          # 128: per-core slice of H (per gate)
VP = -(-V // NCORES)      # 6283: vocab rows per core
VPP = 6656                # padded vocab rows per core (= 4 * 1664)
QW = 1664                 # W_out SBUF tile width (quarter of VPP)
NQ = 4
CHUNKS = [(0, 512), (512, 512), (1024, 512), (1536, 128)]  # within a quarter
NCH = NQ * len(CHUNKS)    # 16 logit psum chunks
WOUT_HOIST = 26           # W_out tiles whose DMAs issue before the phases
PAD_BIAS = -1.0e4         # logit bias for padded vocab slots (exp -> 0)

_CACHE = {}


def _build_bass():
    import concourse.bass as bass  # noqa: F401
    import concourse.mybir as mybir
    import concourse.tile as tile
    from concourse import bacc
    from concourse.masks import make_identity

    F32 = mybir.dt.float32
    BF16 = mybir.dt.bfloat16
    AF = mybir.ActivationFunctionType
    ALU = mybir.AluOpType
    RG = [list(range(NCORES))]

    nc = bacc.Bacc("TRN2", target_bir_lowering=False, debug=False,
                   num_devices=NCORES)

    # ---- I/O ----
    ain_d = nc.dram_tensor("ain_p", [128, 16], F32, kind="ExternalInput")
    h0_d = nc.dram_tensor("h0_p", [128, 8], F32, kind="ExternalInput")
    h1_d = nc.dram_tensor("h1_p", [128, 8], F32, kind="ExternalInput")
    enc = nc.dram_tensor("enc", [L, H], F32, kind="ExternalInput")
    wattn = nc.dram_tensor("wattn", [16, 128, L], F32, kind="ExternalInput")
    wcomb = nc.dram_tensor("wcomb", [16, 128, HS], F32, kind="ExternalInput")
    battn = nc.dram_tensor("battn", [128, 2], F32, kind="ExternalInput")
    bcomb = nc.dram_tensor("bcomb", [HS], F32, kind="ExternalInput")
    lstmw0 = nc.dram_tensor("lstmw0", [8, 128, 1024], F32, kind="ExternalInput")
    lstmw1 = nc.dram_tensor("lstmw1", [8, 128, 1024], F32, kind="ExternalInput")
    lstmb = nc.dram_tensor("lstmb", [2 * 512], F32, kind="ExternalInput")
    c_sl = nc.dram_tensor("c_sl", [2 * HS], F32, kind="ExternalInput")
    woutt = nc.dram_tensor("woutt", [H, VPP], BF16, kind="ExternalInput")
    bout = nc.dram_tensor("bout", [VPP], F32, kind="ExternalInput")

    lp_out = nc.dram_tensor("lp_out", [VPP], F32, kind="ExternalOutput")
    h_out = nc.dram_tensor("h_out", [2 * HS], F32, kind="ExternalOutput")
    c_out = nc.dram_tensor("c_out", [2 * HS], F32, kind="ExternalOutput")
    aw_out = nc.dram_tensor("aw_out", [L], F32, kind="ExternalOutput")
    warm_out = nc.dram_tensor("warm_out", [NCORES], F32, kind="ExternalOutput")

    def row1(ap1d):
        """flat DRAM vector viewed as [1, n] (single partition)."""
        return ap1d.rearrange("(j f) -> j f", j=1)

    with tile.TileContext(nc) as tc:
        with (
            tc.tile_pool(name="sb", bufs=1) as sb,
            tc.tile_pool(name="wap", bufs=1) as wap,
            tc.tile_pool(name="wcp", bufs=1) as wcp,
            tc.tile_pool(name="encp", bufs=1) as encp,
            tc.tile_pool(name="lstmp", bufs=4) as lstmp,
            tc.tile_pool(name="woutp", bufs=WOUT_HOIST) as woutp,
            tc.tile_pool(name="boutp", bufs=2) as boutp,
            tc.tile_pool(name="escp", bufs=2) as escp,
            tc.tile_pool(name="pss", bufs=3, space="PSUM") as pss,
            tc.tile_pool(name="psl", bufs=4, space="PSUM") as psl,
            tc.tile_pool(name="dram", bufs=1, space="DRAM") as dram,
        ):
            # ---- constants ----
            ones_col = sb.tile([128, 1], F32)
            nc.vector.memset(ones_col, 1.0)
            ones_row = sb.tile([1, 128], F32)
            nc.vector.memset(ones_row, 1.0)
            ident = sb.tile([128, 128], F32)
            make_identity(nc, ident[:])

            # ---- warm-up AllGather: absorbs ncfw cold-start latency ----
            wb = dram.tile([1], F32, name="wb")
            nc.scalar.dma_start(row1(wb), ones_col[0:1, 0:1])
            wg = dram.tile([NCORES], F32, name="wg", addr_space="Shared")
            nc.gpsimd.collective_compute(
                "AllGather", ALU.bypass, replica_groups=RG,
                ins=[wb.opt()], outs=[wg.opt()],
            )

            # ---- small input loads (one DMA each) ----
            ain = sb.tile([128, 16], F32)
            nc.sync.dma_start(ain, ain_d.ap())
            h0f = sb.tile([128, 8], F32)
            nc.sync.dma_start(h0f, h0_d.ap())
            h1f = sb.tile([128, 8], F32)
            nc.sync.dma_start(h1f, h1_d.ap())
            battn_sb = sb.tile([128, 2], F32)
            nc.sync.dma_start(battn_sb, battn.ap())
            bcombr = sb.tile([1, HS], F32)
            nc.sync.dma_start(bcombr, row1(bcomb.ap()))
            lstmbr = sb.tile([1, 1024], F32)
            nc.sync.dma_start(lstmbr, row1(lstmb.ap()))
            cslr = sb.tile([1, 256], F32)
            nc.sync.dma_start(cslr, row1(c_sl.ap()))
            wattn_sb = wap.tile([128, 16, L], F32)
            nc.sync.dma_start(wattn_sb, wattn.ap().rearrange("k p f -> p k f"))
            enc_sb = encp.tile([128, 2, 1024], F32)
            nc.sync.dma_start(enc_sb, enc.ap().rearrange("(c p) f -> p c f",
                                                         p=128))
            wcomb_sb = wcp.tile([128, 16, HS], F32)
            nc.sync.dma_start(wcomb_sb, wcomb.ap().rearrange("k p f -> p k f"))

            # ---- LSTM weight loads (4 tiles x 2 layers, one DMA each) ----
            lw_dram = [lstmw0, lstmw1]
            lw_sb = [[], []]
            for ly in range(2):
                for t in range(4):
                    lt = lstmp.tile([128, 2, 1024], F32, name="lw_t")
                    nc.sync.dma_start(
                        lt, lw_dram[ly].ap()[2 * t:2 * t + 2].rearrange(
                            "k p f -> p k f"))
                    lw_sb[ly].append(lt)

            def lw_ih(ly, kc):      # w_ih.T slice chunk [128, 512]
                return lw_sb[ly][kc // 2][:, kc % 2, 0:512]

            def lw_hh(ly, kc):      # w_hh.T slice chunk [128, 512]
                return lw_sb[ly][kc // 2][:, kc % 2, 512:1024]

            # ---- hoisted W_out stream (no deps; fills DMA queues early) ----
            wo_tiles = []
            for i in range(WOUT_HOIST):
                q, kc = divmod(i, 8)
                wt = woutp.tile([128, QW], BF16, name="wout_t")
                nc.sync.dma_start(
                    wt, woutt.ap()[kc * 128:(kc + 1) * 128, q * QW:(q + 1) * QW])
                wo_tiles.append(wt)

            # ---- attention scores (weights-stationary; scores on partitions)
            ps_sc = pss.tile([128, 8], F32, name="ps_sc", tag="pss_t")
            for mb in range(2):
                for kc in range(16):
                    nc.tensor.matmul(
                        ps_sc[:, mb:mb + 1],
                        wattn_sb[:, kc, mb * 128:(mb + 1) * 128],
                        ain[:, kc:kc + 1],
                        start=(kc == 0), stop=(kc == 15),
                    )
            sc = sb.tile([128, 2], F32)
            nc.vector.tensor_add(sc, ps_sc[:, 0:2], battn_sb)
            # softmax over 256 scores (no max-subtraction: scores are O(1))
            esc = sb.tile([128, 2], F32)
            rowsum = sb.tile([128, 1], F32)
            nc.scalar.activation(esc, sc, AF.Exp, accum_out=rowsum)
            zps = pss.tile([1, 1], F32, name="zps", tag="pss_t")
            nc.tensor.matmul(zps, rowsum, ones_col, start=True, stop=True)
            rz = sb.tile([1, 1], F32)
            nc.vector.reciprocal(rz, zps)

            # ---- attn_applied (unnormalized) = exp_scores @ encoder ----
            ps_app = pss.tile([128, 8], F32, name="ps_app", tag="pss_t")
            for hb in range(8):
                for lc in range(2):
                    nc.tensor.matmul(
                        ps_app[:, hb:hb + 1],
                        enc_sb[:, lc, hb * 128:(hb + 1) * 128],
                        esc[:, lc:lc + 1],
                        start=(lc == 0), stop=(lc == 1),
                    )
            appn = sb.tile([128, 8], F32)
            nc.vector.tensor_copy(appn, ps_app)

            # ---- x slice = relu(W_comb_sl @ [embedded, applied/Z] + b) ----
            psa = psl.tile([1, 512], F32, name="ps_log")
            for kc in range(8):
                nc.tensor.matmul(psa[:, :HS], ain[:, kc:kc + 1],
                                 wcomb_sb[:, kc, :],
                                 start=(kc == 0), stop=(kc == 7))
            psb = psl.tile([1, 512], F32, name="ps_log")
            for kc in range(8):
                nc.tensor.matmul(psb[:, :HS], appn[:, kc:kc + 1],
                                 wcomb_sb[:, kc + 8, :],
                                 start=(kc == 0), stop=(kc == 7))
            xbn = sb.tile([1, HS], F32)
            nc.vector.tensor_scalar_mul(xbn, psb[:, :HS], rz)
            xt = sb.tile([1, HS], F32)
            nc.vector.tensor_add(xt, xbn, psa[:, :HS])
            xt2 = sb.tile([1, HS], F32)
            nc.vector.tensor_add(xt2, xt, bcombr)
            xrow = sb.tile([1, HS], F32)
            nc.vector.tensor_scalar_max(xrow, xt2, 0.0)

            # attn_weights output (off the critical path)
            bc_ps = pss.tile([128, 1], F32, name="bc_ps", tag="pss_t")
            nc.tensor.matmul(bc_ps, ones_row, rz, start=True, stop=True)
            bc = sb.tile([128, 1], F32)
            nc.vector.tensor_copy(bc, bc_ps)
            awn = sb.tile([128, 2], F32)
            nc.vector.tensor_scalar_mul(awn, esc, bc)
            nc.scalar.dma_start(aw_out.ap().rearrange("(j p) -> p j", p=128), awn)

            def gather_to_cols(name, src_row, dtype):
                """AllGather a [1,128] row; return full vector as [128,8] cols."""
                b = dram.tile([HS], F32, name=f"{name}_b")
                nc.sync.dma_start(row1(b), src_row)
                g = dram.tile([H], F32, name=f"{name}_g", addr_space="Shared")
                nc.gpsimd.collective_compute(
                    "AllGather", ALU.bypass, replica_groups=RG,
                    ins=[b.opt()], outs=[g.opt()],
                )
                g8 = sb.tile([8, 128], F32, name=f"{name}_g8")
                nc.sync.dma_start(g8, g.rearrange("(c p) -> c p", c=8))
                tp = pss.tile([128, 8], F32, name=f"{name}_tp", tag="pss_t")
                nc.tensor.transpose(tp, g8, ident[:8, :8])
                cols = sb.tile([128, 8], dtype, name=f"{name}_cols")
                nc.vector.tensor_copy(cols, tp)
                return cols

            # ---- LSTM h-side gate matmuls: independent of x / gathered h,
            # so they run during the attention phase and AllGather waits ----
            pg = []
            for ly in range(2):
                h_chunks = h0f if ly == 0 else h1f
                p = psl.tile([1, 512], F32, name="ps_log")
                for kc in range(8):
                    nc.tensor.matmul(p, h_chunks[:, kc:kc + 1], lw_hh(ly, kc),
                                     start=(kc == 0), stop=False)
                pg.append(p)

            # ---- LSTM layers (x-stationary: gates as [1,512] rows) ----
            x_chunks = gather_to_cols("x", xrow, F32)
            for ly in range(2):
                for kc in range(8):
                    nc.tensor.matmul(pg[ly], x_chunks[:, kc:kc + 1],
                                     lw_ih(ly, kc),
                                     start=False, stop=(kc == 7))
                gsum = sb.tile([1, 512], F32, name=f"gsum{ly}")
                nc.vector.tensor_add(gsum, pg[ly], lstmbr[:, ly * 512:(ly + 1) * 512])
                gi, gf, gg, go = (gsum[:, 128 * k:128 * (k + 1)] for k in range(4))
                sigi = sb.tile([1, HS], F32, name=f"sigi{ly}")
                sigf = sb.tile([1, HS], F32, name=f"sigf{ly}")
                tg = sb.tile([1, HS], F32, name=f"tg{ly}")
                sigo = sb.tile([1, HS], F32, name=f"sigo{ly}")
                nc.scalar.activation(sigi, gi, AF.Sigmoid)
                nc.scalar.activation(sigf, gf, AF.Sigmoid)
                nc.scalar.activation(tg, gg, AF.Tanh)
                nc.scalar.activation(sigo, go, AF.Sigmoid)
                t1 = sb.tile([1, HS], F32, name=f"t1_{ly}")
                t2 = sb.tile([1, HS], F32, name=f"t2_{ly}")
                cnew = sb.tile([1, HS], F32, name=f"cnew{ly}")
                nc.vector.tensor_mul(t1, sigf, cslr[:, ly * 128:(ly + 1) * 128])
                nc.vector.tensor_mul(t2, sigi, tg)
                nc.vector.tensor_add(cnew, t1, t2)
                tanhc = sb.tile([1, HS], F32, name=f"tanhc{ly}")
                nc.scalar.activation(tanhc, cnew, AF.Tanh)
                hnew = sb.tile([1, HS], F32, name=f"hnew{ly}")
                nc.vector.tensor_mul(hnew, sigo, tanhc)

                nc.scalar.dma_start(row1(c_out.ap()[ly * HS:(ly + 1) * HS]), cnew)
                nc.scalar.dma_start(row1(h_out.ap()[ly * HS:(ly + 1) * HS]), hnew)
                x_chunks = gather_to_cols(f"h{ly}", hnew,
                                          F32 if ly == 0 else BF16)

            x1 = x_chunks  # full h of layer 1 (bf16), [128, 8] columns

            # ---- logits = x1 @ W_out.T + b_out (vocab-sharded) ----
            lg = sb.tile([1, VPP], F32)
            sums = sb.tile([1, NCH], F32)
            ch = 0
            for q in range(NQ):
                for kc in range(8):
                    i = q * 8 + kc
                    if i >= WOUT_HOIST:
                        wt = woutp.tile([128, QW], BF16, name="wout_t")
                        nc.sync.dma_start(
                            wt, woutt.ap()[kc * 128:(kc + 1) * 128,
                                           q * QW:(q + 1) * QW])
                        wo_tiles.append(wt)
                for off, w in CHUNKS:
                    pl = psl.tile([1, 512], F32, name="ps_log")
                    for kc in range(8):
                        nc.tensor.matmul(
                            pl[:, :w], x1[:, kc:kc + 1],
                            wo_tiles[q * 8 + kc][:, off:off + w],
                            start=(kc == 0), stop=(kc == 7),
                        )
                    go = q * QW + off
                    bo = boutp.tile([1, 512], F32, name="bout_t")
                    nc.scalar.dma_start(bo[:, :w], row1(bout.ap()[go:go + w]))
                    nc.vector.tensor_add(lg[:, go:go + w], pl[:, :w], bo[:, :w])
                    escr = escp.tile([1, 512], F32, name="escr")
                    nc.scalar.activation(escr[:, :w], lg[:, go:go + w], AF.Exp,
                                         accum_out=sums[:, ch:ch + 1])
                    ch += 1
            assert ch == NCH

            # ---- global log-softmax denominator + final subtract ----
            sumtot = sb.tile([1, 1], F32)
            nc.vector.reduce_sum(sumtot, sums, axis=mybir.AxisListType.X)
            seb = dram.tile([1], F32, name="seb")
            nc.sync.dma_start(row1(seb), sumtot)
            seg = dram.tile([NCORES], F32, name="seg", addr_space="Shared")
            nc.gpsimd.collective_compute(
                "AllGather", ALU.bypass, replica_groups=RG,
                ins=[seb.opt()], outs=[seg.opt()],
            )
            ses = sb.tile([1, NCORES], F32)
            nc.sync.dma_start(ses, row1(seg))
            ztot = sb.tile([1, 1], F32)
            nc.vector.reduce_sum(ztot, ses, axis=mybir.AxisListType.X)
            lnz = sb.tile([1, 1], F32)
            nc.scalar.activation(lnz, ztot, AF.Ln)
            nlnz = sb.tile([1, 1], F32)
            nc.vector.tensor_scalar_mul(nlnz, lnz, -1.0)
            for ch in range(NCH):
                q, ci = divmod(ch, len(CHUNKS))
                off, w = CHUNKS[ci]
                s = lg[:, q * QW + off:q * QW + off + w]
                if ch % 2 == 0:
                    nc.scalar.activation(s, s, AF.Identity, bias=nlnz)
                else:
                    nc.vector.tensor_scalar_add(s, s, nlnz)
            nc.sync.dma_start(row1(lp_out.ap()), lg)
            nc.scalar.dma_start(warm_out.ap(), wg)

    nc.compile()
    return nc


def get_nc():
    if "nc" not in _CACHE:
        _CACHE["nc"] = _build_bass()
    return _CACHE["nc"]


def make_in_maps(inputs):
    import ml_dtypes
    f32 = np.float32
    bf16 = ml_dtypes.bfloat16
    inp = {k: np.asarray(v) for k, v in inputs.items()}
    emb = inp["emb"].astype(f32)
    hidden = inp["hidden"].astype(f32)
    cell = inp["cell"].astype(f32)
    idx = int(np.asarray(inp["features"]).ravel()[0])

    embedded = emb[idx]                                   # [H]
    h0 = hidden[0, 0]
    h1 = hidden[1, 0]
    attn_in = np.concatenate([embedded, h0])              # [2H]
    ain_p = np.ascontiguousarray(attn_in.reshape(16, 128).T)   # [128, 16] f32
    h0_p = np.ascontiguousarray(h0.reshape(8, 128).T)          # [128, 8] f32
    h1_p = np.ascontiguousarray(h1.reshape(8, 128).T)          # [128, 8] f32

    wattn_t = inp["W_attn"].astype(f32).T                 # [2H, L]
    wattn_c = np.ascontiguousarray(wattn_t.reshape(16, 128, L))
    comb_t = inp["W_comb"].astype(f32).T                  # [2H, H]
    battn_p = np.ascontiguousarray(
        inp["b_attn"].astype(f32).reshape(2, 128).T)                # [128, 2]
    enc = np.ascontiguousarray(inp["encoder_outputs"].astype(f32))

    # padded W_out.T / b_out
    Wp = np.zeros((NCORES * VP, H), f32)
    Wp[:V] = inp["W_out"].astype(f32)
    bp = np.full(NCORES * VP, PAD_BIAS, f32)
    bp[:V] = inp["b_out"].astype(f32)

    in_maps = []
    for c in range(NCORES):
        rows = np.concatenate(
            [g * H + c * HS + np.arange(HS) for g in range(4)])  # gate slices
        lw = []
        lb = np.zeros(2 * 512, f32)
        for ly in range(2):
            wih_t = inp[f"w_ih_l{ly}"].astype(f32)[rows].T       # [H, 512]
            whh_t = inp[f"w_hh_l{ly}"].astype(f32)[rows].T       # [H, 512]
            lw.append(np.ascontiguousarray(
                np.concatenate([wih_t, whh_t], axis=1)).reshape(8, 128, 1024))
            bsum = (inp[f"b_ih_l{ly}"].astype(f32)
                    + inp[f"b_hh_l{ly}"].astype(f32))
            lb[ly * 512:(ly + 1) * 512] = bsum[rows]

        wsl = Wp[c * VP:(c + 1) * VP]                            # [VP, H]
        wout_t = np.zeros((H, VPP), f32)
        wout_t[:, :VP] = wsl.T
        bout_c = np.full(VPP, PAD_BIAS, f32)
        bout_c[:VP] = bp[c * VP:(c + 1) * VP]

        in_maps.append({
            "ain_p": ain_p,
            "h0_p": h0_p,
            "h1_p": h1_p,
            "enc": enc,
            "wattn": wattn_c,
            "wcomb": np.ascontiguousarray(
                comb_t[:, c * HS:(c + 1) * HS].reshape(16, 128, HS)),
            "battn": battn_p,
            "bcomb": np.ascontiguousarray(
                inp["b_comb"].astype(f32)[c * HS:(c + 1) * HS]),
            "lstmw0": lw[0],
            "lstmw1": lw[1],
            "lstmb": lb,
            "c_sl": np.ascontiguousarray(np.concatenate(
                [cell[0, 0, c * HS:(c + 1) * HS],
                 cell[1, 0, c * HS:(c + 1) * HS]])),
            "woutt": wout_t.astype(bf16),
            "bout": bout_c,
        })
    return in_maps


def assemble_outputs(results):
    results = [{k: np.asarray(v).reshape(-1) for k, v in r.items()}
               for r in results]
    lp = np.concatenate([r["lp_out"][:VP] for r in results])[:V]
    h0 = np.concatenate([r["h_out"][:HS] for r in results])
    h1 = np.concatenate([r["h_out"][HS:] for r in results])
    c0 = np.concatenate([r["c_out"][:HS] for r in results])
    c1 = np.concatenate([r["c_out"][HS:] for r in results])
    hidden_out = np.stack([h0, h1])[:, None, :]
    cell_out = np.stack([c0, c1])[:, None, :]
    aw = results[0]["aw_out"][None, :]
    return lp[None, :], (hidden_out, cell_out), aw


def run_on_hw(inputs, trace=False):
    from concourse.bass_utils import run_bass_kernel_spmd
    nc = get_nc()
    in_maps = make_in_maps(inputs)
    res = run_bass_kernel_spmd(nc, in_maps, list(range(NCORES)), trace=trace)
    return assemble_outputs(res.results), res


def kernel(**inputs):
    outputs, _ = run_on_hw(inputs, trace=False)
    return outputs
